# revision 31
# baseline (speedup 1.0000x reference)
"""Trainium2 Bass kernel for nn_AdvResNet (dense_mlp, 8 NeuronCores).

Reference math (adv=1 path, the one setup_inputs produces):
    beta_norm[n] = sum_k |beta[k, n]|                       # [1024]
    one[n]      = 4096 * sum_h W2[n, h] + bias2[n]          # [1024]
    out[b, n]   = (x @ beta)[b, n] + bias_lin[n]
                  - 0.1 * y[b, n] * beta_norm[n] + one[n]

The x@W1 relu MLP is dead code when adv=1, so W1/bias1 never touch the
device.

Distribution: a 2 (n-halves) x 4 (batch-quarters) grid with ZERO
collectives — collective_compute costs ~73us of latency in this
environment (measured on a bare 8KB AllReduce), far more than the extra
DMA this layout pays.  Core c = (h = c%2, g = c//2) computes
outT[h-half n (512), batch-quarter g (1024 b)].  Every per-n quantity
is then core-local: beta_norm from the core's own beta[:, n-half]
(abs-accumulated on ACT/DVE while the matmul streams), and one[n] from
the core's own W2[n-half, :] rows (free-axis vector reduce).

Compute is in TRANSPOSED layout: outT = beta^T @ x^T via
matmul(psum[n,b], lhsT=beta[k,n] (natural layout), rhs=xT[k,b]), so the
per-n vectors (beta_norm, one, biases) are per-partition scalars, which
feed the scalar-engine activation(scale*in+bias) directly.

Matmuls run in float32r (fp32 operands, 1 cycle/row at N=512).
"""

import os
import sys

sys.path.insert(0, "/opt/trn_rl_repo")
os.environ.setdefault("NEURON_RT_RESET_CORES", "1")

import numpy as np

import concourse.bass as bass  # noqa: F401
import concourse.tile as tile
from concourse import masks
from concourse import bacc, mybir
from concourse.bass_utils import run_bass_kernel_spmd

B, NIN, NHID, NOUT = 4096, 2048, 4096, 1024
NC = 8
PN, PB = 2, 4  # core grid: n-halves x batch-quarters
NH = NOUT // PN  # 512 n per core
BSH = B // PB  # 1024 batch rows per core
KT = NIN // 128  # 16 k-tiles
NT = NH // 128  # 4 n-tiles per core
W2C = 8  # W2 h-chunks streamed per core (chunk-major packed)
EPS = 0.1
F32 = mybir.dt.float32
F32R = mybir.dt.float32r

_CACHE = {}


def build_bass():
    nc = bacc.Bacc("TRN2", target_bir_lowering=False, debug=False, num_devices=NC)

    xT = nc.declare_dram_parameter("xT", [NIN, BSH], F32, isOutput=False)
    yT = nc.declare_dram_parameter("yT", [NH, BSH], F32, isOutput=False)
    bet = nc.declare_dram_parameter("beta", [NIN, NH], F32, isOutput=False)
    w2p = nc.declare_dram_parameter("w2p", [W2C, 128, NT, NHID // W2C], F32, isOutput=False)
    blp = nc.declare_dram_parameter("blp", [128, NT], F32, isOutput=False)
    b2p = nc.declare_dram_parameter("b2p", [128, NT], F32, isOutput=False)
    out = nc.declare_dram_parameter("out", [NH, BSH], F32, isOutput=True)

    HC = NHID // W2C

    with (
        tile.TileContext(nc) as tc,
        tc.tile_pool(name="bsb", bufs=KT) as bpool,
        tc.tile_pool(name="xsb", bufs=KT) as xpool,
        tc.tile_pool(name="yts", bufs=NT) as ypool,
        tc.tile_pool(name="absb", bufs=2) as apool,
        tc.tile_pool(name="w2b", bufs=2) as wpool,
        tc.tile_pool(name="aux", bufs=1) as aux,
        tc.tile_pool(name="psum", bufs=1, space="PSUM") as ppool,
        tc.tile_pool(name="dram", bufs=1, space="DRAM") as dpool,
    ):
        U32 = mybir.dt.uint32
        ps = [
            [
                ppool.tile([128, 512], F32, name=f"ps{t}_{j}", tag=f"ps{t}_{j}")
                for j in range(2)
            ]
            for t in range(NT)
        ]
        acc = aux.tile([128, NH], F32)
        w2acc = aux.tile([128, NT], F32)
        ident = aux.tile([128, 128], F32)
        masks.make_identity(nc, ident[:])

        # scalar(ACT) queue carries ONLY DMA issues (+ the 4 late yt
        # activations) — compute on it would stall behind DMA-ring
        # backpressure.  W2 + yT + biases ride the scalar HWDGE ring.
        blt = aux.tile([128, NT], F32)
        nc.scalar.dma_start(out=blt[:], in_=blp[:])
        b2t = aux.tile([128, NT], F32)
        nc.scalar.dma_start(out=b2t[:], in_=b2p[:])
        wts = []
        for c in range(W2C):
            wt = wpool.tile([128, NT, HC], F32, tag="wt", name=f"wt{c}")
            nc.scalar.dma_start(out=wt[:], in_=w2p[c])
            wts.append(wt)
        yts = []
        for t in range(NT):
            yt = ypool.tile([128, BSH], F32, tag="yt", name=f"yt{t}")
            nc.scalar.dma_start(out=yt[:], in_=yT[t * 128 : (t + 1) * 128, :])
            yts.append(yt)

        # beta tiles on the sync ring; the beta_norm chain (|.| via u32
        # AND-mask + accumulate) runs on the otherwise-idle gpsimd engine,
        # paced by beta arrivals.
        bts = []
        xts = []

        def beta_step(k):
            bt = bpool.tile([128, NH], F32R, tag="bt", name=f"bt{k}")
            nc.sync.dma_start(
                out=bt[:], in_=bet[k * 128 : (k + 1) * 128, :].bitcast(F32R)
            )
            bts.append(bt)
            ab = apool.tile([128, NH], U32, tag="ab")
            nc.vector.tensor_scalar(
                ab[:], bt[:].bitcast(U32), 0x7FFFFFFF, None,
                op0=mybir.AluOpType.bitwise_and,
            )
            if k == 0:
                nc.vector.tensor_copy(acc[:], ab[:].bitcast(F32))
            else:
                nc.vector.tensor_add(acc[:], acc[:], ab[:].bitcast(F32))

        def x_step(k):
            xt = xpool.tile([128, BSH], F32R, tag="xt", name=f"xt{k}")
            nc.sync.dma_start(
                out=xt[:], in_=xT[k * 128 : (k + 1) * 128, :].bitcast(F32R)
            )
            xts.append(xt)
            for t in range(NT):
                for j in range(2):
                    if (t, j) == (0, 0):
                        if k < 4:
                            continue  # catch-up MMs appended after the stream
                        st, sp = (k == 4), False
                    else:
                        st, sp = (k == 0), (k == KT - 1)
                    nc.tensor.matmul(
                        ps[t][j][:],
                        lhsT=bts[k][:, t * 128 : (t + 1) * 128],
                        rhs=xt[:, j * 512 : (j + 1) * 512],
                        start=st,
                        stop=sp,
                    )

        # W2 reduces on DVE, interleaved by emission so the DVE FIFO never
        # head-blocks scale (which the yt activations need) behind the last
        # W2 chunk.
        def w2_reduce(c):
            pr = aux.tile([128, NT], F32, name=f"w2pr{c}", tag=f"w2pr{c}")
            nc.vector.tensor_reduce(
                out=pr[:],
                in_=wts[c][:],
                axis=mybir.AxisListType.X,
                op=mybir.AluOpType.add,
            )
            if c == 0:
                nc.vector.tensor_copy(w2acc[:], pr[:])
            else:
                nc.vector.tensor_add(w2acc[:], w2acc[:], pr[:])

        for k in range(12):
            beta_step(k)

        for k in range(4):
            x_step(k)
            beta_step(12 + k)

        # beta_norm partition-reduce WITHOUT any DMA (DMA-path ops starve
        # until the streams drain): PE-transpose acc in 4 [128,128] blocks
        # into a psum tile that time-shares ps[0][0]'s slot (released before
        # the first matmul needs it), then one free-axis DVE reduce lands
        # beta_norm directly as per-partition columns.
        bnp = ppool.tile([128, NT, 128], F32, tag="ps0_0", name="bnp")
        for c in range(NT):
            nc.tensor.transpose(
                bnp[:, c, :], acc[:, c * 128 : (c + 1) * 128], ident[:]
            )
        bnc = aux.tile([128, NT], F32)
        nc.vector.tensor_reduce(
            out=bnc[:], in_=bnp[:], axis=mybir.AxisListType.X,
            op=mybir.AluOpType.add,
        )
        scale = aux.tile([128, NT], F32)
        nc.vector.tensor_scalar_mul(scale[:], bnc[:], -EPS)
        for c in range(W2C):
            w2_reduce(c)
        biasc = aux.tile([128, NT], F32)
        nc.vector.tensor_scalar_mul(biasc[:], w2acc[:], float(NHID))
        nc.vector.tensor_add(biasc[:], biasc[:], b2t[:])
        nc.vector.tensor_add(biasc[:], biasc[:], blt[:])

        # t[n] = yT*scale + biasc on ACT, all prerequisites land mid-stream.
        for t in range(NT):
            nc.scalar.activation(
                yts[t][:],
                yts[t][:],
                mybir.ActivationFunctionType.Identity,
                bias=biasc[:, t : t + 1],
                scale=scale[:, t : t + 1],
            )

        for k in range(4, KT):
            x_step(k)
        # ps[0][0] catch-up: k-tiles 0-3 from the resident beta/xT tiles.
        for k2 in range(4):
            nc.tensor.matmul(
                ps[0][0][:],
                lhsT=bts[k2][:, 0:128],
                rhs=xts[k2][:, 0:512],
                start=False,
                stop=(k2 == 3),
            )

        # Epilogue: out = psum + t (DVE, with two adds offloaded to the idle
        # gpsimd), then store on the drained rings (alternating).
        pairs = [(t, j) for t in range(NT) for j in range(2) if (t, j) != (0, 0)]
        pairs.append((0, 0))
        for t, j in pairs:
            sl = slice(j * 512, (j + 1) * 512)
            nc.vector.tensor_add(yts[t][:, sl], ps[t][j][:], yts[t][:, sl])
            eng = nc.sync if j == 0 else nc.scalar
            eng.dma_start(
                out=out[t * 128 : (t + 1) * 128, sl], in_=yts[t][:, sl]
            )

    nc.compile()
    return nc


def _get_nc():
    if "nc" not in _CACHE:
        _CACHE["nc"] = build_bass()
    return _CACHE["nc"]


def _shard_inputs(x, y, beta, bias_lin, W2, bias2):
    x = np.ascontiguousarray(x, dtype=np.float32)
    y = np.ascontiguousarray(y, dtype=np.float32)
    beta = np.ascontiguousarray(beta, dtype=np.float32)
    W2 = np.ascontiguousarray(W2, dtype=np.float32)
    bias_lin = np.asarray(bias_lin, np.float32)
    bias2 = np.asarray(bias2, np.float32)
    xT_full = np.ascontiguousarray(x.T)  # [NIN, B]
    xT_g = [
        np.ascontiguousarray(xT_full[:, g * BSH : (g + 1) * BSH]) for g in range(PB)
    ]
    beta_h = [
        np.ascontiguousarray(beta[:, h * NH : (h + 1) * NH]) for h in range(PN)
    ]
    HC = NHID // W2C
    w2p_h = []
    for h in range(PN):
        base = W2[h * NH : (h + 1) * NH, :].reshape(NT, 128, NHID).transpose(1, 0, 2)
        w2p_h.append(
            np.ascontiguousarray(
                np.stack([base[:, :, c * HC : (c + 1) * HC] for c in range(W2C)])
            )
        )
    blp_h = [
        np.ascontiguousarray(bias_lin[h * NH : (h + 1) * NH].reshape(NT, 128).T)
        for h in range(PN)
    ]
    b2p_h = [
        np.ascontiguousarray(bias2[h * NH : (h + 1) * NH].reshape(NT, 128).T)
        for h in range(PN)
    ]
    in_maps = []
    for c in range(NC):
        h, g = c % PN, c // PN
        yT = np.ascontiguousarray(
            y[g * BSH : (g + 1) * BSH, h * NH : (h + 1) * NH].T
        )
        in_maps.append(
            {
                "xT": xT_g[g],
                "yT": yT,
                "beta": beta_h[h],
                "w2p": w2p_h[h],
                "blp": blp_h[h],
                "b2p": b2p_h[h],
            }
        )
    return in_maps


def run_device(inputs, trace=False, **kw):
    nc = _get_nc()
    in_maps = _shard_inputs(
        inputs["x"], inputs["y"], inputs["beta"], inputs["bias_lin"],
        inputs["W2"], inputs["bias2"],
    )
    res = run_bass_kernel_spmd(nc, in_maps, core_ids=list(range(NC)), trace=trace, **kw)
    full = np.empty((B, NOUT), dtype=np.float32)
    for c in range(NC):
        h, g = c % PN, c // PN
        full[g * BSH : (g + 1) * BSH, h * NH : (h + 1) * NH] = res.results[c][
            "out"
        ].T
    return full, res


def _reference_numpy(x, y, beta, bias_lin, W1, W2, bias1, bias2, adv):
    # Fallback for the adv=0 path (never produced by setup_inputs).
    x = np.asarray(x, np.float32)
    lin = x @ np.asarray(beta, np.float32) + np.asarray(bias_lin, np.float32)
    if adv:
        beta_norm = np.sum(np.abs(np.asarray(beta, np.float32)), axis=0)
        lin = lin - EPS * np.asarray(y, np.float32) * beta_norm
        one = NHID * np.sum(np.asarray(W2, np.float32), axis=1) + np.asarray(
            bias2, np.float32
        )
        one = np.broadcast_to(one, lin.shape)
    else:
        h = np.maximum(
            x @ np.asarray(W1, np.float32).T + np.asarray(bias1, np.float32), 0.0
        )
        one = h @ np.asarray(W2, np.float32).T + np.asarray(bias2, np.float32)
    return (lin + one).astype(np.float32)


def kernel(**inputs) -> np.ndarray:
    adv = int(np.asarray(inputs.get("adv", 1)))
    if adv == 0:
        return _reference_numpy(
            inputs["x"], inputs["y"], inputs["beta"], inputs["bias_lin"],
            inputs["W1"], inputs["W2"], inputs["bias1"], inputs["bias2"], adv,
        )
    full, _ = run_device(inputs)
    return full


# revision 32
# speedup vs baseline: 1.0125x; 1.0125x over previous
"""Trainium2 Bass kernel for nn_AdvResNet (dense_mlp, 8 NeuronCores).

Reference math (adv=1 path, the one setup_inputs produces):
    beta_norm[n] = sum_k |beta[k, n]|                       # [1024]
    one[n]      = 4096 * sum_h W2[n, h] + bias2[n]          # [1024]
    out[b, n]   = (x @ beta)[b, n] + bias_lin[n]
                  - 0.1 * y[b, n] * beta_norm[n] + one[n]

The x@W1 relu MLP is dead code when adv=1, so W1/bias1 never touch the
device.

Distribution: a 2 (n-halves) x 4 (batch-quarters) grid with ZERO
collectives — collective_compute costs ~73us of latency in this
environment (measured on a bare 8KB AllReduce), far more than the extra
DMA this layout pays.  Core c = (h = c%2, g = c//2) computes
outT[h-half n (512), batch-quarter g (1024 b)].  Every per-n quantity
is then core-local: beta_norm from the core's own beta[:, n-half]
(abs-accumulated on ACT/DVE while the matmul streams), and one[n] from
the core's own W2[n-half, :] rows (free-axis vector reduce).

Compute is in TRANSPOSED layout: outT = beta^T @ x^T via
matmul(psum[n,b], lhsT=beta[k,n] (natural layout), rhs=xT[k,b]), so the
per-n vectors (beta_norm, one, biases) are per-partition scalars, which
feed the scalar-engine activation(scale*in+bias) directly.

Matmuls run in float32r (fp32 operands, 1 cycle/row at N=512).
"""

import os
import sys

sys.path.insert(0, "/opt/trn_rl_repo")
os.environ.setdefault("NEURON_RT_RESET_CORES", "1")

import numpy as np

import concourse.bass as bass  # noqa: F401
import concourse.tile as tile
from concourse import masks
from concourse import bacc, mybir
from concourse.bass_utils import run_bass_kernel_spmd

B, NIN, NHID, NOUT = 4096, 2048, 4096, 1024
NC = 8
PN, PB = 2, 4  # core grid: n-halves x batch-quarters
NH = NOUT // PN  # 512 n per core
BSH = B // PB  # 1024 batch rows per core
KT = NIN // 128  # 16 k-tiles
NT = NH // 128  # 4 n-tiles per core
W2C = 8  # W2 h-chunks streamed per core (chunk-major packed)
EPS = 0.1
F32 = mybir.dt.float32
F32R = mybir.dt.float32r

_CACHE = {}


def build_bass():
    nc = bacc.Bacc("TRN2", target_bir_lowering=False, debug=False, num_devices=NC)

    xT = nc.declare_dram_parameter("xT", [NIN, BSH], F32, isOutput=False)
    yT = nc.declare_dram_parameter("yT", [NH, BSH], F32, isOutput=False)
    bet = nc.declare_dram_parameter("beta", [NIN, NH], F32, isOutput=False)
    w2p = nc.declare_dram_parameter("w2p", [W2C, 128, NT, NHID // W2C], F32, isOutput=False)
    blp = nc.declare_dram_parameter("blp", [128, NT], F32, isOutput=False)
    b2p = nc.declare_dram_parameter("b2p", [128, NT], F32, isOutput=False)
    out = nc.declare_dram_parameter("out", [NH, BSH], F32, isOutput=True)

    HC = NHID // W2C

    with (
        tile.TileContext(nc) as tc,
        tc.tile_pool(name="bsb", bufs=KT) as bpool,
        tc.tile_pool(name="xsb", bufs=KT) as xpool,
        tc.tile_pool(name="yts", bufs=NT) as ypool,
        tc.tile_pool(name="absb", bufs=2) as apool,
        tc.tile_pool(name="w2b", bufs=2) as wpool,
        tc.tile_pool(name="aux", bufs=1) as aux,
        tc.tile_pool(name="psum", bufs=1, space="PSUM") as ppool,
        tc.tile_pool(name="dram", bufs=1, space="DRAM") as dpool,
    ):
        U32 = mybir.dt.uint32
        ps = [
            [
                ppool.tile([128, 512], F32, name=f"ps{t}_{j}", tag=f"ps{t}_{j}")
                for j in range(2)
            ]
            for t in range(NT)
        ]
        acc = aux.tile([128, NH], F32)
        w2acc = aux.tile([128, NT], F32)
        ident = aux.tile([128, 128], F32)
        masks.make_identity(nc, ident[:])

        # scalar(ACT) queue carries ONLY DMA issues (+ the 4 late yt
        # activations) — compute on it would stall behind DMA-ring
        # backpressure.  W2 + yT + biases ride the scalar HWDGE ring.
        blt = aux.tile([128, NT], F32)
        nc.scalar.dma_start(out=blt[:], in_=blp[:])
        b2t = aux.tile([128, NT], F32)
        nc.scalar.dma_start(out=b2t[:], in_=b2p[:])
        wts = []
        for c in range(W2C):
            wt = wpool.tile([128, NT, HC], F32, tag="wt", name=f"wt{c}")
            nc.scalar.dma_start(out=wt[:], in_=w2p[c])
            wts.append(wt)
        yts = []
        for t in range(NT):
            yt = ypool.tile([128, BSH], F32, tag="yt", name=f"yt{t}")
            nc.scalar.dma_start(out=yt[:], in_=yT[t * 128 : (t + 1) * 128, :])
            yts.append(yt)

        # beta tiles on the sync ring; the beta_norm chain (|.| via u32
        # AND-mask + accumulate) runs on the otherwise-idle gpsimd engine,
        # paced by beta arrivals.
        bts = []
        xts = []

        def beta_step(k):
            bt = bpool.tile([128, NH], F32R, tag="bt", name=f"bt{k}")
            nc.sync.dma_start(
                out=bt[:], in_=bet[k * 128 : (k + 1) * 128, :].bitcast(F32R)
            )
            bts.append(bt)
            ab = apool.tile([128, NH], U32, tag="ab")
            nc.vector.tensor_scalar(
                ab[:], bt[:].bitcast(U32), 0x7FFFFFFF, None,
                op0=mybir.AluOpType.bitwise_and,
            )
            if k == 0:
                nc.vector.tensor_copy(acc[:], ab[:].bitcast(F32))
            else:
                nc.vector.tensor_add(acc[:], acc[:], ab[:].bitcast(F32))

        def x_step(k):
            xt = xpool.tile([128, BSH], F32R, tag="xt", name=f"xt{k}")
            nc.sync.dma_start(
                out=xt[:], in_=xT[k * 128 : (k + 1) * 128, :].bitcast(F32R)
            )
            xts.append(xt)
            for t in range(NT):
                for j in range(2):
                    if (t, j) == (0, 0):
                        if k < 4:
                            continue  # catch-up MMs appended after the stream
                        st, sp = (k == 4), False
                    else:
                        st, sp = (k == 0), (k == KT - 1)
                    nc.tensor.matmul(
                        ps[t][j][:],
                        lhsT=bts[k][:, t * 128 : (t + 1) * 128],
                        rhs=xt[:, j * 512 : (j + 1) * 512],
                        start=st,
                        stop=sp,
                    )

        # W2 reduces on DVE, interleaved by emission so the DVE FIFO never
        # head-blocks scale (which the yt activations need) behind the last
        # W2 chunk.
        def w2_reduce(c):
            pr = aux.tile([128, NT], F32, name=f"w2pr{c}", tag=f"w2pr{c}")
            nc.vector.tensor_reduce(
                out=pr[:],
                in_=wts[c][:],
                axis=mybir.AxisListType.X,
                op=mybir.AluOpType.add,
            )
            if c == 0:
                nc.vector.tensor_copy(w2acc[:], pr[:])
            else:
                nc.vector.tensor_add(w2acc[:], w2acc[:], pr[:])

        for k in range(12):
            beta_step(k)

        for k in range(4):
            x_step(k)
            beta_step(12 + k)

        # beta_norm partition-reduce WITHOUT any DMA (DMA-path ops starve
        # until the streams drain): PE-transpose acc in 4 [128,128] blocks
        # into a psum tile that time-shares ps[0][0]'s slot (released before
        # the first matmul needs it), then one free-axis DVE reduce lands
        # beta_norm directly as per-partition columns.
        bnp = ppool.tile([128, NT, 128], F32, tag="ps0_0", name="bnp")
        for c in range(NT):
            nc.tensor.transpose(
                bnp[:, c, :], acc[:, c * 128 : (c + 1) * 128], ident[:]
            )
        bnc = aux.tile([128, NT], F32)
        nc.vector.tensor_reduce(
            out=bnc[:], in_=bnp[:], axis=mybir.AxisListType.X,
            op=mybir.AluOpType.add,
        )
        scale = aux.tile([128, NT], F32)
        nc.vector.tensor_scalar_mul(scale[:], bnc[:], -EPS)
        for c in range(W2C):
            w2_reduce(c)
        biasc = aux.tile([128, NT], F32)
        nc.vector.tensor_scalar_mul(biasc[:], w2acc[:], float(NHID))
        nc.vector.tensor_add(biasc[:], biasc[:], b2t[:])
        nc.vector.tensor_add(biasc[:], biasc[:], blt[:])

        # t[n] = yT*scale + biasc on ACT, all prerequisites land mid-stream.
        for t in range(NT):
            nc.scalar.activation(
                yts[t][:],
                yts[t][:],
                mybir.ActivationFunctionType.Identity,
                bias=0.0,
                scale=scale[:, t : t + 1],
            )

        for k in range(4, KT):
            x_step(k)
        # ps[0][0] catch-up: k-tiles 0-3 from the resident beta/xT tiles.
        for k2 in range(4):
            nc.tensor.matmul(
                ps[0][0][:],
                lhsT=bts[k2][:, 0:128],
                rhs=xts[k2][:, 0:512],
                start=False,
                stop=(k2 == 3),
            )

        # Epilogue: out = psum + t (DVE, with two adds offloaded to the idle
        # gpsimd), then store on the drained rings (alternating).
        pairs = [(t, j) for t in range(NT) for j in range(2) if (t, j) != (0, 0)]
        pairs.append((0, 0))
        for t, j in pairs:
            sl = slice(j * 512, (j + 1) * 512)
            nc.vector.scalar_tensor_tensor(
                out=yts[t][:, sl],
                in0=ps[t][j][:],
                scalar=biasc[:, t : t + 1],
                in1=yts[t][:, sl],
                op0=mybir.AluOpType.add,
                op1=mybir.AluOpType.add,
            )
            eng = nc.sync if j == 0 else nc.scalar
            eng.dma_start(
                out=out[t * 128 : (t + 1) * 128, sl], in_=yts[t][:, sl]
            )

    nc.compile()
    return nc


def _get_nc():
    if "nc" not in _CACHE:
        _CACHE["nc"] = build_bass()
    return _CACHE["nc"]


def _shard_inputs(x, y, beta, bias_lin, W2, bias2):
    x = np.ascontiguousarray(x, dtype=np.float32)
    y = np.ascontiguousarray(y, dtype=np.float32)
    beta = np.ascontiguousarray(beta, dtype=np.float32)
    W2 = np.ascontiguousarray(W2, dtype=np.float32)
    bias_lin = np.asarray(bias_lin, np.float32)
    bias2 = np.asarray(bias2, np.float32)
    xT_full = np.ascontiguousarray(x.T)  # [NIN, B]
    xT_g = [
        np.ascontiguousarray(xT_full[:, g * BSH : (g + 1) * BSH]) for g in range(PB)
    ]
    beta_h = [
        np.ascontiguousarray(beta[:, h * NH : (h + 1) * NH]) for h in range(PN)
    ]
    HC = NHID // W2C
    w2p_h = []
    for h in range(PN):
        base = W2[h * NH : (h + 1) * NH, :].reshape(NT, 128, NHID).transpose(1, 0, 2)
        w2p_h.append(
            np.ascontiguousarray(
                np.stack([base[:, :, c * HC : (c + 1) * HC] for c in range(W2C)])
            )
        )
    blp_h = [
        np.ascontiguousarray(bias_lin[h * NH : (h + 1) * NH].reshape(NT, 128).T)
        for h in range(PN)
    ]
    b2p_h = [
        np.ascontiguousarray(bias2[h * NH : (h + 1) * NH].reshape(NT, 128).T)
        for h in range(PN)
    ]
    in_maps = []
    for c in range(NC):
        h, g = c % PN, c // PN
        yT = np.ascontiguousarray(
            y[g * BSH : (g + 1) * BSH, h * NH : (h + 1) * NH].T
        )
        in_maps.append(
            {
                "xT": xT_g[g],
                "yT": yT,
                "beta": beta_h[h],
                "w2p": w2p_h[h],
                "blp": blp_h[h],
                "b2p": b2p_h[h],
            }
        )
    return in_maps


def run_device(inputs, trace=False, **kw):
    nc = _get_nc()
    in_maps = _shard_inputs(
        inputs["x"], inputs["y"], inputs["beta"], inputs["bias_lin"],
        inputs["W2"], inputs["bias2"],
    )
    res = run_bass_kernel_spmd(nc, in_maps, core_ids=list(range(NC)), trace=trace, **kw)
    full = np.empty((B, NOUT), dtype=np.float32)
    for c in range(NC):
        h, g = c % PN, c // PN
        full[g * BSH : (g + 1) * BSH, h * NH : (h + 1) * NH] = res.results[c][
            "out"
        ].T
    return full, res


def _reference_numpy(x, y, beta, bias_lin, W1, W2, bias1, bias2, adv):
    # Fallback for the adv=0 path (never produced by setup_inputs).
    x = np.asarray(x, np.float32)
    lin = x @ np.asarray(beta, np.float32) + np.asarray(bias_lin, np.float32)
    if adv:
        beta_norm = np.sum(np.abs(np.asarray(beta, np.float32)), axis=0)
        lin = lin - EPS * np.asarray(y, np.float32) * beta_norm
        one = NHID * np.sum(np.asarray(W2, np.float32), axis=1) + np.asarray(
            bias2, np.float32
        )
        one = np.broadcast_to(one, lin.shape)
    else:
        h = np.maximum(
            x @ np.asarray(W1, np.float32).T + np.asarray(bias1, np.float32), 0.0
        )
        one = h @ np.asarray(W2, np.float32).T + np.asarray(bias2, np.float32)
    return (lin + one).astype(np.float32)


def kernel(**inputs) -> np.ndarray:
    adv = int(np.asarray(inputs.get("adv", 1)))
    if adv == 0:
        return _reference_numpy(
            inputs["x"], inputs["y"], inputs["beta"], inputs["bias_lin"],
            inputs["W1"], inputs["W2"], inputs["bias1"], inputs["bias2"], adv,
        )
    full, _ = run_device(inputs)
    return full
